# revision 35
# baseline (speedup 1.0000x reference)
"""Trainium2 Bass kernel for nn_BatchedFCN (batched ensemble MLP + max).

Reference computation (per network n of 1024, batch B=256):
    h = relu(x @ W1_n^T + b1); h = relu(h @ W2_n^T + b2); h = relu(h @ W3_n^T + b3)
    h = relu(h @ W4_n^T + b4); y_n = h @ W5_n^T + b5          # [B, 1]
    out[b] = max_n y_n[b]                                      # [B]

Sharding: the 1024 networks are split across 8 NeuronCores (128 nets/core).
Each core computes a partial max over its networks; the host folds the 8
partial results.

Per-core dataflow (activations transposed [features, batch]):
  L1 runs on the PE in fp8 DoubleRow perf mode (0.5 cycles/row). Accuracy is
  recovered with a 3-term hi/lo decomposition executed as 6 accumulating
  DoubleRow matmuls per net:
      16*h1 = Wh*xh + Wh*xl + Wl*xr
  where W=16*W1', Wh=e4m3(W), Wl=e4m3(16*(W-Wh)), xh=e4m3(x), xl=e4m3(x-xh),
  xr=e4m3(x/16).  The act1 evacuation applies relu with scale=1/16 on DVE
  (a mult+max tensor_scalar; ACT is saturated by the mid evacs and any act1
  there bursts the step).
  L2/L3 are bf16 form-1 matmuls writing into a shared 2-bank PSUM tile
  M(t) = [p2(pair t-2) | p3(pair t-4)]; ONE fused 1024-wide ACT relu
  evacuation per step empties both, halving per-instruction overhead (only
  the valid half at the pipeline edges, shortening fill/drain).
  L4 is form-2 (h3 slice stationary, augmented W4'' moving, 52 cols: 50 w4
  features + b5-ones column + pad); two consecutive pairs share one
  single-bank PSUM tile so p4 lands batch-major [128b, 2pr, 2h, 2j, 52].
  The tail is exact: m4 = relu(p4) * w5 via one 2-pair-fused DVE
  scalar_tensor_tensor against a broadcast w5 tile, a Pool pairwise
  pre-fold halves the width, and a DVE add-reduce writes y straight into
  the [128, 256] staging tile (the last 4 pairs run an unfused per-pair
  tail without the Pool hop to shorten the drain).  The per-net max is
  folded on the host together with the 8-core fold.

All weight DMAs are emitted once, up front, in global deadline order on the
SP ring (x on the ACT ring): with 2 weight groups and double-buffered pools
no DMA ever waits on a buffer, so the (single-slot) DMA engine streams
continuously from t=0.  A handful of warm-up matmuls on a zeroed fp8 tile
ramp the PE clock while the first weights are in flight.

Weight/bias augmentation: one extra input row (bias) and one ones-column
propagate biases through every layer with no separate bias operands.
"""

import sys

import numpy as np

try:
    import concourse  # noqa: F401
except ImportError:  # fall back to the container's staged repo
    sys.path.insert(0, "/opt/trn_rl_repo")

import ml_dtypes  # noqa: E402

import concourse.mybir as mybir  # noqa: E402
import concourse.tile as tile  # noqa: E402
from concourse import bacc, bass_utils  # noqa: E402

# Problem shapes (hardcoded per contract)
NN = 1024  # total networks
B = 256  # batch
NCORES = 8
NPC = NN // NCORES  # networks per core = 128
PAIRS = NPC // 2  # 64
GROUPS = 2  # weight-DMA groups per core
GNETS = NPC // GROUPS  # 64 nets per group
GPAIRS = GNETS // 2  # 32 pairs per group

KA = 501  # augmented L1 contraction (500 inputs + bias row)
MA = 101  # augmented hidden width (100 + ones column)
MP = 112  # L1 per-k-tile output group, padded for dual-fp8 16B alignment
W1N = 4 * 2 * MP  # w1 cols per net = 896 (Wh_c0 | Wh_c1 | Wl_c0 | Wl_c1)
P0, P1 = 126, 125  # k-tile partition counts: c0 = feats 0..251, c1 = 252..500
M4C = 52  # L4'' cols per net: 50 w4 | b5-ones | pad
S1 = 16.0  # L1 fp8 weight scale
NWARM = 30  # PE clock warm-up matmuls (cover the ~3us DMA startup)

BF16 = ml_dtypes.bfloat16
E4M3 = ml_dtypes.float8_e4m3

_PROGRAM_CACHE = {}


def _build_program():
    nc = bacc.Bacc("TRN2", debug=False, num_devices=NCORES)
    f8 = mybir.dt.float8e4
    bf = mybir.dt.bfloat16
    f32 = mybir.dt.float32

    xp_d = nc.dram_tensor("xp", [128, 3072], f8, kind="ExternalInput").ap()
    w1_d = nc.dram_tensor("w1p", [128, NPC * W1N], f8, kind="ExternalInput").ap()
    w2_d = nc.dram_tensor("w2p", [MA, NPC * MA], bf, kind="ExternalInput").ap()
    w3_d = nc.dram_tensor("w3p", [MA, NPC * MA], bf, kind="ExternalInput").ap()
    w4_d = nc.dram_tensor("w4p", [MA, NPC * M4C], bf, kind="ExternalInput").ap()
    w5_d = nc.dram_tensor("w5b", [128, NPC * M4C], bf, kind="ExternalInput").ap()
    out_d = nc.dram_tensor("out", [128, 256], f32, kind="ExternalOutput").ap()

    relu = mybir.ActivationFunctionType.Relu
    DR = mybir.MatmulPerfMode.DoubleRow
    mult = mybir.AluOpType.mult
    amax = mybir.AluOpType.max
    aadd = mybir.AluOpType.add

    # L1 matmul schedule: (k-partitions, w col offset within net, x col offset)
    L1MM = [
        (P0, 0, 0),  # Wh_c0 x xh_c0
        (P1, 2 * MP, 512),  # Wh_c1 x xh_c1
        (P0, 0, 1024),  # Wh_c0 x xl_c0
        (P1, 2 * MP, 1536),  # Wh_c1 x xl_c1
        (P0, 4 * MP, 2048),  # Wl_c0 x xr_c0
        (P1, 6 * MP, 2560),  # Wl_c1 x xr_c1
    ]

    def r3(ap):
        return ap.rearrange("p (two m) -> p two m", two=2)

    with tile.TileContext(nc) as tc:
        from contextlib import ExitStack

        with ExitStack() as ctx:
            consts = ctx.enter_context(tc.tile_pool(name="consts", bufs=1))
            wp1 = ctx.enter_context(tc.tile_pool(name="wp1", bufs=1))
            wp2 = ctx.enter_context(tc.tile_pool(name="wp2", bufs=1))
            wp3 = ctx.enter_context(tc.tile_pool(name="wp3", bufs=1))
            wp4 = ctx.enter_context(tc.tile_pool(name="wp4", bufs=1))
            wp5 = ctx.enter_context(tc.tile_pool(name="wp5", bufs=1))
            hp = ctx.enter_context(tc.tile_pool(name="hp", bufs=3))
            # PSUM: p1 [128,512]x2 (2 banks) + mid quad [128,1024]x2
            # (4 banks) + p4 [128,208]x2 (2 banks) = 8 banks exactly
            pp = ctx.enter_context(tc.tile_pool(name="pp", bufs=2, space="PSUM"))

            # ---------------- constants + warm-up ----------------
            xp = consts.tile([128, 3072], f8)
            # y staging: col = half*128 + 2*pair + netloc, written once/pair
            y_all = consts.tile([128, 256], f32)
            # zeroed fp8 operand for PE clock warm-up matmuls
            wz = consts.tile([128, 512], f8)
            nc.vector.memset(wz, 0.0)
            # trigger the one-time ACT table load immediately
            warm = consts.tile([1, 2], f32)
            nc.vector.memset(warm, 0.0)
            nc.scalar.activation(warm[0:1, 1:2], warm[0:1, 0:1], relu)

            # ---------------- weight tiles (all resident) ----------------
            group_tiles = []
            for g in range(GROUPS):
                w1t = wp1.tile([128, GNETS * W1N], f8, name=f"w1t{g}")
                w2t = wp2.tile([MA, GNETS * MA], bf, name=f"w2t{g}")
                w3t = wp3.tile([MA, GNETS * MA], bf, name=f"w3t{g}")
                w4t = wp4.tile([MA, GNETS * M4C], bf, name=f"w4t{g}")
                w5t = wp5.tile([128, GNETS * M4C], bf, name=f"w5t{g}")
                group_tiles.append((w1t, w2t, w3t, w4t, w5t))

            # ---------------- upfront deadline-ordered DMA program --------
            # deadline = first step (pair index) that reads the chunk
            chunks = []

            def add(dl, prio, fn):
                chunks.append((dl, prio, len(chunks), fn))

            def mk_chunk(wt, wd, nw, g, n0, n1):
                def fn(wt=wt, wd=wd, nw=nw, g=g, n0=n0, n1=n1):
                    nc.sync.dma_start(
                        wt[:, n0 * nw : n1 * nw],
                        wd[:, (g * GNETS + n0) * nw : (g * GNETS + n1) * nw],
                    )

                return fn

            W1B = [(0, 2), (2, 4), (4, 6), (6, 8), (8, 12), (12, 16),
                   (16, 20), (20, 24), (24, 28), (28, 32), (32, 36), (36, 40),
                   (40, 46), (46, 52), (52, 58), (58, 64)]
            WMB = [(0, 8), (8, 16), (16, 32), (32, 48), (48, 64)]
            for g in range(GROUPS):
                pg = g * GPAIRS
                w1t, w2t, w3t, w4t, w5t = group_tiles[g]
                for a, b in W1B:
                    add(pg + a // 2, 0, mk_chunk(w1t, w1_d, W1N, g, a, b))
                for a, b in WMB:
                    add(pg + a // 2 + 2, 1, mk_chunk(w2t, w2_d, MA, g, a, b))
                for a, b in WMB:
                    add(pg + a // 2 + 4, 2, mk_chunk(w3t, w3_d, MA, g, a, b))
                for a, b in WMB:
                    add(pg + a // 2 + 6, 3, mk_chunk(w4t, w4_d, M4C, g, a, b))
                for a, b in WMB:
                    add(pg + a // 2 + 7, 4, mk_chunk(w5t, w5_d, M4C, g, a, b))

            chunks.sort(key=lambda c: (c[0], c[1], c[2]))
            # first w1 chunk, then x (ACT ring), then the rest in order
            chunks[0][3]()
            nc.scalar.dma_start(xp, xp_d)
            for _, _, _, fn in chunks[1:]:
                fn()

            # ---------------- PE warm-up while DMAs land ----------------
            pwarm = pp.tile([128, 512], f32, tag="p1", name="pwarm")
            for _ in range(NWARM):
                nc.tensor.matmul(
                    pwarm[0:MP, 0:B],
                    lhsT=r3(wz[0:P0, 0 : 2 * MP]),
                    rhs=r3(wz[0:P0, 0:512]),
                    perf_mode=DR,
                )

            # Software pipeline over pairs p:
            #   L1@p  act1@p+1  L2@p+2  midevac@p+3  L3@p+4  midevac@p+5
            #   L4@p+6  stt@p+7  reduce@p+8
            SKEW_MAX = 9
            p1_t, m2_t, p4_t = {}, {}, {}
            h1_t, h2_t, h3_t, m4_t = {}, {}, {}, {}

            def loc(p):
                # group-local A/B net indices for pair p
                jj = p % GPAIRS
                return p // GPAIRS, 2 * jj, 2 * jj + 1

            for t in range(PAIRS + SKEW_MAX):
                # ---- shared 2-bank mid PSUM tile: L2(pair t-2) -> cols
                # 0:512, L3(pair t-4) -> cols 512:1024
                l2p, l3p = t - 2, t - 4
                if 0 <= l2p < PAIRS or 0 <= l3p < PAIRS:
                    m2_t[t] = pp.tile([128, 1024], f32, tag="pmid", name="Mq")

                # ---- PE stage L2 (pair t-2)
                p_ = l2p
                if 0 <= p_ < PAIRS:
                    g, nA, nB_ = loc(p_)
                    w2t = group_tiles[g][1]
                    h1 = h1_t.pop(p_)
                    p2 = m2_t[t]
                    for nl, fo in ((nA, 0), (nB_, B)):
                        nc.tensor.matmul(
                            p2[0:MA, fo : fo + B],
                            lhsT=w2t[:, nl * MA : (nl + 1) * MA],
                            rhs=h1[:, fo : fo + B],
                        )

                # ---- PE stage L3 (pair t-4): upper half of the mid tile
                p_ = l3p
                if 0 <= p_ < PAIRS:
                    g, nA, nB_ = loc(p_)
                    w3t = group_tiles[g][2]
                    h2 = h2_t.pop(p_)
                    p3 = m2_t[t]
                    for nl, fo in ((nA, 2 * B), (nB_, 3 * B)):
                        nc.tensor.matmul(
                            p3[0:MA, fo : fo + B],
                            lhsT=w3t[:, nl * MA : (nl + 1) * MA],
                            rhs=h2[:, fo - 2 * B : fo - B],
                        )

                # ---- PE stage L4'' (pair t-6): form-2, h3 stationary.
                # Two consecutive pairs share one [128, 2*4*M4C] PSUM tile
                # (1664B < one bank) so the stt/reduce tail can process both
                # in single fused DVE instructions.
                p_ = t - 6
                if 0 <= p_ < PAIRS:
                    g, nA, nB_ = loc(p_)
                    w4t = group_tiles[g][3]
                    h3 = h3_t.pop(p_)
                    if p_ % 2 == 0:
                        p4 = pp.tile([128, 8 * M4C], f32, tag="p4", name="p4q")
                        p4_t[p_ // 2] = p4
                    else:
                        p4 = p4_t[p_ // 2]
                    # col layout [h, pr, j, i]: h outermost so the w5
                    # broadcast AP in the stt canonicalizes to 3D
                    pr = p_ % 2
                    for j, nl in enumerate((nA, nB_)):
                        for h in range(2):
                            po = (h * 4 + pr * 2 + j) * M4C
                            nc.tensor.matmul(
                                p4[0:128, po : po + M4C],
                                lhsT=h3[
                                    0:MA, j * B + h * 128 : j * B + (h + 1) * 128
                                ],
                                rhs=w4t[:, nl * M4C : (nl + 1) * M4C],
                            )

                # ---- PE stage L1 (pair t): 6 DoubleRow matmuls per net
                p_ = t
                if 0 <= p_ < PAIRS:
                    g, nA, nB_ = loc(p_)
                    w1t = group_tiles[g][0]
                    p1 = pp.tile([128, 512], f32, tag="p1")
                    for nl, fo in ((nA, 0), (nB_, B)):
                        for j, (P, wo, xo) in enumerate(L1MM):
                            nc.tensor.matmul(
                                p1[0:MP, fo : fo + B],
                                lhsT=r3(
                                    w1t[0:P, nl * W1N + wo : nl * W1N + wo + 2 * MP]
                                ),
                                rhs=r3(xp[0:P, xo : xo + 512]),
                                start=(j == 0),
                                stop=(j == len(L1MM) - 1),
                                perf_mode=DR,
                            )
                    p1_t[p_] = p1

                # ---- act1 (pair t-1): h1 = relu(p1/16) on DVE. ACT is
                # saturated by the mid evacs; putting any act1 on ACT makes
                # that step burst ~1650ns and the ripple costs 1-3us.
                p_ = t - 1
                if 0 <= p_ < PAIRS:
                    p1 = p1_t.pop(p_)
                    h1 = hp.tile([MA, 512], bf, tag="h1")
                    nc.vector.tensor_scalar(
                        h1, p1[0:MA, :], 1.0 / S1, 0.0, mult, amax
                    )
                    h1_t[p_] = h1

                # ---- fused mid evac (ACT): relu of M(t-1) = [p2(t-3) |
                # p3(t-5)] in one 1024-wide instruction; at the pipeline
                # edges only the valid half is evacuated (halves the
                # latency-bound fill/drain chain)
                if (t - 1) in m2_t:
                    m2 = m2_t.pop(t - 1)
                    hm = hp.tile([MA, 1024], bf, tag="hm")
                    lo = 0 <= t - 3 < PAIRS
                    hi = 0 <= t - 5 < PAIRS
                    if lo and hi:
                        nc.scalar.activation(hm, m2[0:MA, :], relu)
                    elif lo:
                        nc.scalar.activation(
                            hm[:, 0:512], m2[0:MA, 0:512], relu
                        )
                    elif hi:
                        nc.scalar.activation(
                            hm[:, 512:1024], m2[0:MA, 512:1024], relu
                        )
                    if lo:
                        h2_t[t - 3] = hm[:, 0:512]
                    if hi:
                        h3_t[t - 5] = hm[:, 512:1024]

                # ---- tail for the LAST 4 pairs: per-pair stt + direct
                # reduce (no Pool hop) — shortens the latency-bound drain
                p_ = t - 7
                if PAIRS - 4 <= p_ < PAIRS:
                    g, nA, nB_ = loc(p_)
                    w5t = group_tiles[g][4]
                    pq = p4_t[p_ // 2]
                    pr = p_ % 2
                    m4 = hp.tile([128, 4 * M4C], bf, tag="m4s", bufs=2)
                    p4v = pq.rearrange(
                        "p (h pr j i) -> p pr h j i", pr=2, h=2, j=2
                    )[:, pr]
                    m4v = m4.rearrange("p (h j i) -> p h j i", h=2, j=2)
                    w5p = (
                        w5t[:, nA * M4C : (nA + 2) * M4C]
                        .rearrange("p (j i) -> p j i", j=2)
                        .unsqueeze(1)
                        .broadcast_to((128, 2, 2, M4C))
                    )
                    nc.vector.scalar_tensor_tensor(m4v, p4v, 0.0, w5p, amax, mult)
                    yo = y_all.rearrange("p (h q) -> p h q", h=2)
                    with nc.allow_low_precision("52-term bf16 product sums"):
                        nc.vector.tensor_reduce(
                            yo[:, :, 2 * p_ : 2 * p_ + 2],
                            m4v,
                            axis=mybir.AxisListType.X,
                            op=aadd,
                        )
                    if pr == 1:
                        p4_t.pop(p_ // 2)

                # ---- stt (pairs t-8, t-7 fused): m4 = relu(p4) * w5 (DVE)
                p_ = t - 7
                if 0 <= p_ < PAIRS - 4 and p_ % 2 == 1:
                    g, nA, nB_ = loc(p_ - 1)
                    w5t = group_tiles[g][4]
                    p4 = p4_t.pop(p_ // 2)
                    m4 = hp.tile([128, 8 * M4C], bf, tag="m4", bufs=2)
                    p4v = p4.rearrange(
                        "p (h pr j i) -> p h pr j i", pr=2, h=2, j=2
                    )
                    m4v = m4.rearrange(
                        "p (h pr j i) -> p h pr j i", pr=2, h=2, j=2
                    )
                    w5p = (
                        w5t[:, nA * M4C : (nA + 4) * M4C]
                        .rearrange("p (pr j i) -> p pr j i", pr=2, j=2)
                        .unsqueeze(1)
                        .broadcast_to((128, 2, 2, 2, M4C))
                    )
                    nc.vector.scalar_tensor_tensor(m4v, p4v, 0.0, w5p, amax, mult)
                    # Pool pre-fold: halve the reduce width on the idle
                    # engine (SBUF-only, so Pool is allowed)
                    m4f = hp.tile([128, 4 * M4C], bf, tag="m4f", bufs=2)
                    with nc.allow_low_precision("bf16 pairwise partial sums"):
                        nc.gpsimd.tensor_tensor(
                            m4f.rearrange("p (b i) -> p b i", b=8),
                            m4v.rearrange("p h pr j i -> p (h pr j) i")[
                                :, :, 0 : M4C // 2
                            ],
                            m4v.rearrange("p h pr j i -> p (h pr j) i")[
                                :, :, M4C // 2 : M4C
                            ],
                            aadd,
                        )
                    m4_t[p_ // 2] = m4f

                # ---- reduce (pairs t-9, t-8 fused): y = sum_i m4f (DVE)
                p_ = t - 8
                if 0 <= p_ < PAIRS - 4 and p_ % 2 == 1:
                    m4f = m4_t.pop(p_ // 2)
                    m4v = m4f.rearrange(
                        "p (h pr j i) -> p h pr j i", pr=2, h=2, j=2
                    )
                    # y_all col = h*128 + 2*pair + j; pair = (p_-1) + pr
                    yo = y_all.rearrange(
                        "p (h q pr j) -> p q h pr j", h=2, pr=2, j=2
                    )
                    with nc.allow_low_precision("52-term bf16 product sums"):
                        nc.vector.tensor_reduce(
                            yo[:, p_ // 2],
                            m4v,
                            axis=mybir.AxisListType.X,
                            op=aadd,
                        )

            # ship the full [128, 256] y staging tile; the host folds the
            # per-net max together with the 8-core fold (saves the device
            # reduce + its serialization from the drain's critical path)
            nc.sync.dma_start(out_d, y_all)

    nc.compile()
    return nc


def _get_program():
    if "nc" not in _PROGRAM_CACHE:
        _PROGRAM_CACHE["nc"] = _build_program()
    return _PROGRAM_CACHE["nc"]


def _q8(a):
    return a.astype(E4M3).astype(np.float32)


def _pack_inputs(inputs):
    """Host-side: transpose, augment, hi/lo-fp8 decompose L1, shard."""
    x = np.asarray(inputs["x"], np.float32)
    w = {i: np.asarray(inputs[f"w{i}"], np.float32) for i in (1, 2, 3, 4, 5)}
    b = {i: np.asarray(inputs[f"b{i}"], np.float32) for i in (1, 2, 3, 4, 5)}

    # ---- L1 operands: augmented xT' = [x^T ; ones] [501, 256]
    xT = np.concatenate([x.T, np.ones((1, B), np.float32)], axis=0)
    xh = _q8(xT)
    xl = _q8(xT - xh)
    xr = _q8(xT / S1)

    def pack_x(xv, base, P):
        # -> [128, 512]: (p, i*256+n) = xv[base + i*P + p, n]
        o = np.zeros((128, 2, B), np.float32)
        nf = min(2 * P, KA - base)
        v = np.zeros((2 * P, B), np.float32)
        v[:nf] = xv[base : base + nf]
        o[0:P] = v.reshape(2, P, B).transpose(1, 0, 2)
        return o.reshape(128, 2 * B)

    xp = np.concatenate(
        [
            pack_x(xh, 0, P0), pack_x(xh, 252, P1),
            pack_x(xl, 0, P0), pack_x(xl, 252, P1),
            pack_x(xr, 0, P0), pack_x(xr, 252, P1),
        ],
        axis=1,
    ).astype(E4M3)  # [128, 3072]

    # ---- W1': [N, 501, 101] scaled by 16, hi/lo e4m3
    W1 = np.zeros((NN, KA, MA), np.float32)
    W1[:, :500, :100] = w[1].transpose(0, 2, 1)
    W1[:, 500, :100] = b[1]
    W1[:, 500, 100] = 1.0
    W1 *= S1
    Wh = _q8(W1)
    Wl = _q8((W1 - Wh) * 16.0)

    def pack_w(Wv, base, P):
        # -> [128, N, 2, MP]: (p, n, i, m) = Wv[n, base + i*P + p, m]
        o = np.zeros((128, NN, 2, MP), np.float32)
        nf = min(2 * P, KA - base)
        v = np.zeros((NN, 2 * P, MA), np.float32)
        v[:, :nf] = Wv[:, base : base + nf]
        o[0:P, :, :, 0:MA] = v.reshape(NN, 2, P, MA).transpose(2, 0, 1, 3)
        return o

    # per net: [4 groups][2 k-tiles][MP]: Wh_c0 | Wh_c1 | Wl_c0 | Wl_c1
    w1p = np.stack(
        [pack_w(Wh, 0, P0), pack_w(Wh, 252, P1), pack_w(Wl, 0, P0), pack_w(Wl, 252, P1)],
        axis=2,
    ).astype(E4M3)  # [128, N, 4, 2, MP]

    def aug_mid(wi, bi):
        # -> [101(part=i), N, 101]; ones-propagation col + bias row folded in
        A = np.zeros((NN, MA, MA), np.float32)
        A[:, :100, :100] = wi.transpose(0, 2, 1)
        A[:, 100, :100] = bi
        A[:, 100, 100] = 1.0
        return A.transpose(1, 0, 2).astype(BF16)

    w2p = aug_mid(w[2], b[2])  # [101, N, 101]
    w3p = aug_mid(w[3], b[3])

    # ---- W4'': [101, N, 52]: 50 unscaled w4 cols (+b4 row) | b5-ones | pad
    A = np.zeros((NN, M4C, MA), np.float32)  # [N, out col, feature]
    A[:, 0:50, :100] = w[4]
    A[:, 0:50, 100] = b[4]
    A[:, 50, 100] = 1.0
    w4p = A.transpose(2, 0, 1).astype(BF16)  # [101, N, 52]

    # ---- w5 broadcast tile [128, N*52]: 50 signed w5 | b5 | 0
    V = np.zeros((NN, M4C), np.float32)
    V[:, 0:50] = w[5][:, 0, :]
    V[:, 50] = b[5][:, 0]
    w5b = np.broadcast_to(
        V.reshape(1, NN, M4C), (128, NN, M4C)
    ).astype(BF16)  # [128, N, 52]

    in_maps = []
    for c in range(NCORES):
        sl = slice(c * NPC, (c + 1) * NPC)
        in_maps.append(
            {
                "xp": xp,
                "w1p": np.ascontiguousarray(
                    w1p[:, sl].reshape(128, NPC * W1N)
                ),
                "w2p": np.ascontiguousarray(w2p[:, sl].reshape(MA, NPC * MA)),
                "w3p": np.ascontiguousarray(w3p[:, sl].reshape(MA, NPC * MA)),
                "w4p": np.ascontiguousarray(w4p[:, sl].reshape(MA, NPC * M4C)),
                "w5b": np.ascontiguousarray(w5b[:, sl].reshape(128, NPC * M4C)),
            }
        )
    return in_maps


def _fold_outputs(results):
    # per-core y staging [128, 256]: col = h*128 + 2*pair + j, batch = h*128+p
    r = np.stack(
        [np.asarray(res["out"], np.float32) for res in results]
    )  # [8, 128, 256]
    m = r.reshape(8, 128, 2, 128).max(axis=(0, 3))  # [128 p, 2 h]
    return np.ascontiguousarray(m.T.reshape(B)).astype(np.float32)


def run(inputs, **run_kwargs):
    """Pack, execute on 8 cores, fold. Returns (output[B], BassKernelResults)."""
    nc = _get_program()
    in_maps = _pack_inputs(inputs)
    res = bass_utils.run_bass_kernel_spmd(
        nc, in_maps, core_ids=list(range(NCORES)), **run_kwargs
    )
    return _fold_outputs(res.results), res


def kernel(**inputs):
    out, _ = run(inputs)
    return out
